# revision 19
# baseline (speedup 1.0000x reference)
"""Trainium2 Bass kernel for 12-head attention (SEQ=4096, D=768), 8-core SPMD.

Sharding: head-parallel with a sequence-split remainder. Core c owns full head
A_c = c and the half of head B_c = 8 + c//2 selected by (c % 2). Upper-half
cores receive a sequence-rolled copy of x so every core's program computes
local queries [0, 2048) for its B head (pure SPMD, no divergent control flow).
Each core returns a partial output projection [768, 4096] (fp16); the host
un-rolls, transposes and sums the 8 partials.

Per-core dataflow (matmuls in fp16, softmax internals in fp32):
  x^T streamed chunk-major -> QKV^T projections (start attention early)
  -> scores S^T[j,i] = K^T(lhsT) x Q^T(rhs) with the pair's two heads on
  disjoint PE row groups -> exp on ScalarE (scale=1/8 folded; scores within
  [-10, 10]) -> attn@V with [V | ones] as stationary operand (denominators
  fall out as PSUM row 64); V reaches [j, d] layout via DMA x-bar transpose
  -> per-pair epilogue overlapped with the next pair's attention: reciprocal
  of the denominator rows straight from PSUM, K=2 broadcast matmul, fused
  drain-normalize (scalar_tensor_tensor), output projection into the freed
  accumulator banks, fp16 DMA out.
"""

import numpy as np

N_CORES = 8
N_HEADS = 12
HEAD_DIM = 64
N_FEATS = 768
SEQ = 4096
FCH = N_FEATS // 128  # contraction chunks of the feature dim
W = 1024              # i-chunk width (exp granularity)
NJB = SEQ // 128      # key blocks
NH = W // 512         # 512-wide matmul sub-chunks per i-chunk
NCH = SEQ // 512      # 512-wide seq chunks

_PROGRAM = None
LAST_RESULT = None


def _build_program():
    import concourse.tile as tile
    from concourse import bacc, mybir

    f32 = mybir.dt.float32
    f32r = mybir.dt.float32r
    f16 = mybir.dt.float16
    EXP = mybir.ActivationFunctionType.Exp
    BYP = mybir.AluOpType.bypass
    MUL = mybir.AluOpType.mult

    nc = bacc.Bacc("TRN2", target_bir_lowering=False, debug=False,
                   num_devices=N_CORES)

    xt_d = nc.dram_tensor("xt", [N_FEATS, SEQ], f16, kind="ExternalInput").ap()
    wqk_d = nc.dram_tensor("wqk", [N_FEATS, 256], f16, kind="ExternalInput").ap()
    wv_d = nc.dram_tensor("wv", [N_FEATS, 128], f16, kind="ExternalInput").ap()
    wo_d = nc.dram_tensor("wo", [128, N_FEATS], f16, kind="ExternalInput").ap()
    sel_d = nc.dram_tensor("sel", [2, 128], f32r, kind="ExternalInput").ap()
    id_d = nc.dram_tensor("ident", [128, 128], f16, kind="ExternalInput").ap()
    out_d = nc.dram_tensor("out", [N_FEATS, SEQ], f16, kind="ExternalOutput").ap()
    import os as _os
    _dbg = _os.environ.get("KDEBUG") == "1"
    if _dbg:
        dbg_vfull = nc.dram_tensor("dbg_vfull", [128, NJB, 2, 128], f16, kind="ExternalOutput").ap()
        dbg_q = nc.dram_tensor("dbg_q", [128, SEQ], f16, kind="ExternalOutput").ap()
        dbg_k = nc.dram_tensor("dbg_k", [128, SEQ], f16, kind="ExternalOutput").ap()
        dbg_dinv = nc.dram_tensor("dbg_dinv", [66, SEQ], f32, kind="ExternalOutput").ap()
        dbg_e = nc.dram_tensor("dbg_e", [128, W], f16, kind="ExternalOutput").ap()

    with tile.TileContext(nc) as tc:
        with tc.tile_pool(name="persist", bufs=1) as pp:
            wqk_sb = pp.tile([128, FCH, 256], f16)
            wv_sb = pp.tile([128, FCH, 128], f16)
            wo_sb = pp.tile([128, N_FEATS], f16)
            sel_sb = pp.tile([66, 128], f32r)
            id_sb = pp.tile([128, 128], f16)
            nc.gpsimd.dma_start(out=id_sb[:], in_=id_d[:])
            nc.gpsimd.dma_start(out=wqk_sb[:], in_=wqk_d.rearrange("(c p) m -> p c m", p=128))
            nc.gpsimd.dma_start(out=wv_sb[:], in_=wv_d.rearrange("(c p) m -> p c m", p=128))
            nc.gpsimd.dma_start(out=wo_sb[:], in_=wo_d[:])
            nc.gpsimd.dma_start(out=sel_sb[64:66, :], in_=sel_d[:])

            # warm the exp activation table while projections run
            scratch = pp.tile([1, 2], f32)
            nc.vector.memset(scratch[:], 0.0)
            nc.scalar.activation(out=scratch[:], in_=scratch[:], func=EXP)

            q_t = [pp.tile([128, 512], f16, name=f"q_t{i}") for i in range(NCH)]
            k_t = [pp.tile([128, 512], f16, name=f"k_t{i}") for i in range(NCH)]
            # rolled copies: A-head data duplicated into rows 64:128 (q only
            # for chunks 6,7 used by the A2 half; k for all chunks)
            q2_t = {i: pp.tile([128, 512], f16, name=f"q2_t{i}") for i in (6, 7)}
            k2_t = [pp.tile([128, 512], f16, name=f"k2_t{i}") for i in range(NCH)]
            vT_t = [pp.tile([128, 512], f16, name=f"vT_t{i}") for i in range(NCH)]
            # V in [j, d] layout: per key block, half A = [v|1] (65 used),
            # half B = [v|1|1] (66 used; double ones row -> den lands on PSUM
            # rows 64:66 so DVE ops keep a 32-aligned partition base)
            vfull = pp.tile([128, NJB, 2, 128], f16)  # halves padded to 256B for the x-bar M2S write path
            nc.gpsimd.memset(vfull[:, :, 0, 64:66], 1.0)
            nc.gpsimd.memset(vfull[:, :, 1, 64:66], 1.0)
            # softmax denominators, rows 64 (A) and 65 (B)
            den = pp.tile([66, SEQ], f32r)
            nc.gpsimd.memset(den[64:66, :].bitcast(f32), 1.0)

            # x^T streamed chunk-major so projections start after ~1/8 of x
            xt = pp.tile([128, FCH, SEQ], f16)
            xt_r = xt_d.rearrange("(c p) n -> p c n", p=128)
            for n in range(NCH):
                for k in range(FCH):
                    nc.gpsimd.dma_start(out=xt[:, k, n * 512:(n + 1) * 512],
                                        in_=xt_r[:, k, n * 512:(n + 1) * 512])

            ps_s_cm = tc.tile_pool(name="ps_s", space="PSUM", bufs=2)
            ps_s = ps_s_cm.__enter__()
            pe_cm = tc.tile_pool(name="exps", bufs=10)
            pe = pe_cm.__enter__()
            ps_ov_cm = tc.tile_pool(name="ps_ov", bufs=1, space="PSUM")
            ps_ov = ps_ov_cm.__enter__()
            p3_cm = tc.tile_pool(name="ph3", bufs=1)
            p3 = p3_cm.__enter__()

            def proj_qk(nch):
                pq = ps_s.tile([128, W], f32, tag="s", name=f"pjqk{nch}")
                for k in range(FCH):
                    nc.tensor.matmul(pq[:, 0:512], wqk_sb[:, k, 0:128],
                                     xt[:, k, nch * 512:(nch + 1) * 512],
                                     start=(k == 0), stop=(k == FCH - 1))
                    nc.tensor.matmul(pq[:, 512:1024], wqk_sb[:, k, 128:256],
                                     xt[:, k, nch * 512:(nch + 1) * 512],
                                     start=(k == 0), stop=(k == FCH - 1))
                nc.vector.tensor_copy(q_t[nch][:], pq[:, 0:512])
                nc.vector.tensor_copy(k_t[nch][:], pq[:, 512:1024])

            def proj_v(nch):
                pv = ps_s.tile([128, 512], f32, tag="s", name=f"pjv{nch}")
                for k in range(FCH):
                    nc.tensor.matmul(pv[:], wv_sb[:, k, :],
                                     xt[:, k, nch * 512:(nch + 1) * 512],
                                     start=(k == 0), stop=(k == FCH - 1))
                nc.vector.tensor_copy(vT_t[nch][:], pv[:])
                ptt = ps_s.tile([128, 4, 128], f16, tag="s", name=f"ptr{nch}")
                for qq in range(4):
                    jb = nch * 4 + qq
                    nc.tensor.transpose(ptt[:, qq, :],
                                        vT_t[nch][:, qq * 128:(qq + 1) * 128], id_sb[:])
                    nc.vector.tensor_copy(vfull[:, jb, 0, 0:64], ptt[:, qq, 0:64])
                    nc.vector.tensor_copy(vfull[:, jb, 1, 0:64], ptt[:, qq, 64:128])

            # chunks 0,1 up front; the rest interleave into pair 0's jb
            # loop (emission order sets scheduler priority - this lets the
            # exp stream start ~10us in instead of after all projections)
            for n in (0, 1):
                proj_qk(n)
                proj_v(n)

            def emit_proj(jb):
                n = jb // 4 + 2
                if jb % 4 == 0 and n < NCH:
                    proj_qk(n)
                    proj_v(n)
                if jb == NJB - 4:
                    # rolled copies for the A2 half (SWDGE; emitted late so
                    # they don't head-of-line-block the xt streaming DMAs)
                    for i in range(NCH):
                        nc.gpsimd.dma_start(out=k2_t[i][64:128, :], in_=k_t[i][0:64, :])
                    for i in (6, 7):
                        nc.gpsimd.dma_start(out=q2_t[i][64:128, :], in_=q_t[i][0:64, :])

            # pair = (head-half c1 on PE rows 0:64, head-half c2 on rows
            # 64:128); each covers a 1024-wide i-chunk
            pairs = [
                (("A", 0, q_t, k_t), ("B", 0, q_t, k_t)),
                (("A", 1, q_t, k_t), ("B", 1, q_t, k_t)),
                (("A", 2, q_t, k_t), ("A2", 3, q2_t, k2_t)),
            ]

            if _dbg:
                for n in range(NCH):
                    nc.gpsimd.dma_start(out=dbg_q[:, n * 512:(n + 1) * 512], in_=q_t[n][:])
                    nc.gpsimd.dma_start(out=dbg_k[:, n * 512:(n + 1) * 512], in_=k_t[n][:])
                nc.gpsimd.dma_start(out=dbg_vfull[:], in_=vfull[:])

            for pi, (c1, c2) in enumerate(pairs):
                ov1 = ps_ov.tile([65, W], f32, tag="ov1", name=f"ov1_{pi}")
                m2 = 66 if c2[0] == "B" else 65
                ov2 = ps_ov.tile([m2, W], f32, tag="ov2", name=f"ov2_{pi}")
                ovs = (ov1, ov2)
                for jb in range(NJB):
                    if pi == 0:
                        emit_proj(jb)
                    jc, jo = jb // 4, (jb % 4) * 128
                    etiles = []
                    for h in range(NH):
                        sp = ps_s.tile([128, W], f32, tag="s", name=f"s{pi}_{jb}_{h}")
                        for ci, (_, ic, qt, kt) in enumerate((c1, c2)):
                            base = ci * 64
                            nc.tensor.matmul(
                                sp[:, ci * 512:(ci + 1) * 512],
                                kt[jc][base:base + 64, jo:jo + 128],
                                qt[ic * NH + h][base:base + 64, :],
                                start=True, stop=True)
                        e = pe.tile([128, W], f16, tag="e", name=f"e{pi}_{jb}_{h}")
                        nc.scalar.activation(out=e[:], in_=sp[:], func=EXP, scale=0.125)
                        if _dbg and pi == 0 and jb == 0 and h == 0:
                            nc.gpsimd.dma_start(out=dbg_e[:], in_=e[:])
                        etiles.append(e)
                    for ci in range(2):
                        half = 0 if (c1, c2)[ci][0].startswith("A") else 1
                        m = ovs[ci].shape[0]
                        for h, e in enumerate(etiles):
                            nc.tensor.matmul(ovs[ci][:, h * 512:(h + 1) * 512],
                                             vfull[:, jb, half, 0:m],
                                             e[:, ci * 512:(ci + 1) * 512],
                                             start=(jb == 0), stop=(jb == NJB - 1))

                # ---- per-pair epilogue, overlapped with next pair ----
                (n1, ic1, _, _), (n2, ic2, _, _) = c1, c2
                p10, p20 = ic1 * W, ic2 * W
                # den rows to SBUF (f32r rounding copy, baseline pattern):
                # for B-pairs rows 64:66 get B first, then A overwrites row 64
                if n2 == "B":
                    nc.vector.tensor_copy(den[64:66, p20:p20 + W], ov2[64:66, :])
                else:
                    nc.vector.tensor_copy(den[64:65, p20:p20 + W], ov2[64:65, :])
                nc.vector.tensor_copy(den[64:65, p10:p10 + W], ov1[64:65, :])

                # column chunks of this pair's i-range: for B-pairs both
                # halves share columns (p10 == p20); for the A/A2 pair the
                # two ranges are distinct with only rows 0:64 populated
                if n2 == "B":
                    chunks = [(p10 + c * 512, ov1, ov2) for c in range(2)]
                else:
                    chunks = [(p10 + c * 512, ov1, None) for c in range(2)] + \
                             [(p20 + c * 512, ov2, None) for c in range(2)]
                # all drains first so the ov accumulator banks release for
                # reuse by the po tiles below (and the next pair's ovs)
                nms = []
                for cb, (c0, ovA, ovB) in enumerate(chunks):
                    lo = c0 - (p10 if ovA is ov1 else p20)
                    bc = ps_s.tile([128, 512], f32, tag="s", name=f"bc{pi}_{cb}")
                    nc.tensor.matmul(bc[:], sel_sb[64:66, :], den[64:66, c0:c0 + 512],
                                     start=True, stop=True)
                    rows = 128 if ovB is not None else 64
                    rc = p3.tile([rows, 512], f32, tag="rc", name=f"rc{pi}_{cb}", bufs=3)
                    nc.vector.reciprocal_approx_fast(out=rc[:], in_=bc[0:rows, :])
                    nm = p3.tile([rows, 512], f16, tag="nm", name=f"nm{pi}_{cb}", bufs=5)
                    nc.vector.scalar_tensor_tensor(
                        out=nm[0:64, :], in0=ovA[0:64, lo:lo + 512], scalar=0.0,
                        in1=rc[0:64, :], op0=BYP, op1=MUL)
                    if ovB is not None:
                        nc.vector.scalar_tensor_tensor(
                            out=nm[64:128, :], in0=ovB[0:64, lo:lo + 512], scalar=0.0,
                            in1=rc[64:128, :], op0=BYP, op1=MUL)
                    nms.append((c0, rows, nm))
                pon = 0
                for cb, (c0, rows, nm) in enumerate(nms):
                    for fb in range(FCH):
                        po = ps_ov.tile([128, 512], f32, tag=("ov1", "ov2")[pon % 2],
                                        name=f"po{pi}_{cb}_{fb}")
                        pon += 1
                        nc.tensor.matmul(po[:], wo_sb[0:rows, fb * 128:(fb + 1) * 128],
                                         nm[:], start=True, stop=True)
                        ob = p3.tile([128, 512], f16, tag="ob", name=f"ob{pi}_{cb}_{fb}",
                                     bufs=4)
                        if pi < 2:
                            nc.vector.tensor_copy(ob[:], po[:])
                        else:
                            nc.scalar.copy(ob[:], po[:])
                        nc.sync.dma_start(out=out_d[fb * 128:(fb + 1) * 128, c0:c0 + 512],
                                          in_=ob[:])

            if _dbg:
                nc.gpsimd.dma_start(out=dbg_dinv[:], in_=den[:].bitcast(f32))
            p3_cm.__exit__(None, None, None)
            ps_ov_cm.__exit__(None, None, None)
            pe_cm.__exit__(None, None, None)
            ps_s_cm.__exit__(None, None, None)

    nc.compile()
    return nc


def _get_program():
    global _PROGRAM
    if _PROGRAM is None:
        _PROGRAM = _build_program()
    return _PROGRAM


def kernel(x: np.ndarray, w_qkv: np.ndarray, w_out: np.ndarray) -> np.ndarray:
    global LAST_RESULT
    import os
    try:
        import antenv.axon_hooks  # noqa: F401
    except ImportError:
        # without the NTFF hook, a leaked BASS_TRACE=1 would crash the
        # axon trace path inside run_bass_kernel_spmd
        os.environ["BASS_NEVER_TRACE"] = "1"
    from concourse.bass_utils import run_bass_kernel_spmd

    nc = _get_program()
    x2 = np.ascontiguousarray(x[0], dtype=np.float32)          # [SEQ, F]
    w_qkv = np.asarray(w_qkv, dtype=np.float32)                # [2304, F]
    w_out = np.asarray(w_out, dtype=np.float32)                # [F, 768]

    # per-head slices of w_qkv rows: o = h*192 + d*3 + {0:q, 1:k, 2:v}
    def wslice(h, which):
        return w_qkv[h * 192 + which:(h + 1) * 192:3, :]       # [64, F]

    sel = np.zeros((2, 128), dtype=np.float32)
    sel[0, 0:64] = 1.0
    sel[1, 64:128] = 1.0
    ident = np.eye(128, dtype=np.float16)

    xt_plain = np.ascontiguousarray(x2.T.astype(np.float16))   # [F, SEQ]
    xt_rolled = np.ascontiguousarray(np.roll(x2, -SEQ // 2, axis=0).T.astype(np.float16))

    in_maps = []
    rolls = []
    for c in range(N_CORES):
        hA = c
        hB = 8 + c // 2
        roll = (SEQ // 2) if (c % 2) else 0
        rolls.append(roll)
        wqk = np.ascontiguousarray(np.concatenate(
            [wslice(hA, 0), wslice(hB, 0), wslice(hA, 1), wslice(hB, 1)],
            axis=0).T.astype(np.float16))
        wv = np.ascontiguousarray(np.concatenate(
            [wslice(hA, 2), wslice(hB, 2)], axis=0).T.astype(np.float16))
        cols = list(range(hA * 64, hA * 64 + 64)) + list(range(hB * 64, hB * 64 + 64))
        wo = np.ascontiguousarray(w_out[:, cols].T.astype(np.float16))  # [128, F]
        in_maps.append({
            "xt": xt_rolled if roll else xt_plain,
            "wqk": wqk, "wv": wv, "wo": wo, "sel": sel, "ident": ident,
        })

    res = run_bass_kernel_spmd(nc, in_maps, list(range(N_CORES)))
    LAST_RESULT = res

    acc = np.zeros((SEQ, N_FEATS), dtype=np.float64)
    for c in range(N_CORES):
        part = res.results[c]["out"].astype(np.float32)        # [F, SEQ]
        if rolls[c]:
            part = np.roll(part, rolls[c], axis=1)
        acc += part.T.astype(np.float64)
    return acc.astype(np.float32)[None]


# revision 22
# speedup vs baseline: 1.0591x; 1.0591x over previous
"""Trainium2 Bass kernel for 12-head attention (SEQ=4096, D=768), 8-core SPMD.

Sharding: head-parallel with a sequence-split remainder. Core c owns full head
A_c = c and the half of head B_c = 8 + c//2 selected by (c % 2). Upper-half
cores receive a sequence-rolled copy of x so every core's program computes
local queries [0, 2048) for its B head (pure SPMD, no divergent control flow).
Each core returns a partial output projection [768, 4096] (fp16); the host
un-rolls, transposes and sums the 8 partials.

Per-core dataflow (matmuls in fp16, softmax internals in fp32):
  x^T streamed chunk-major -> QKV^T projections (start attention early)
  -> scores S^T[j,i] = K^T(lhsT) x Q^T(rhs) with the pair's two heads on
  disjoint PE row groups -> exp on ScalarE (scale=1/8 folded; scores within
  [-10, 10]) -> attn@V with [V | ones] as stationary operand (denominators
  fall out as PSUM row 64); V reaches [j, d] layout via DMA x-bar transpose
  -> per-pair epilogue overlapped with the next pair's attention: reciprocal
  of the denominator rows straight from PSUM, K=2 broadcast matmul, fused
  drain-normalize (scalar_tensor_tensor), output projection into the freed
  accumulator banks, fp16 DMA out.
"""

import numpy as np

N_CORES = 8
N_HEADS = 12
HEAD_DIM = 64
N_FEATS = 768
SEQ = 4096
FCH = N_FEATS // 128  # contraction chunks of the feature dim
W = 1024              # i-chunk width (exp granularity)
NJB = SEQ // 128      # key blocks
NH = W // 512         # 512-wide matmul sub-chunks per i-chunk
NCH = SEQ // 512      # 512-wide seq chunks

_PROGRAM = None
LAST_RESULT = None


def _build_program():
    import concourse.tile as tile
    from concourse import bacc, mybir

    f32 = mybir.dt.float32
    f32r = mybir.dt.float32r
    f16 = mybir.dt.float16
    EXP = mybir.ActivationFunctionType.Exp
    BYP = mybir.AluOpType.bypass
    MUL = mybir.AluOpType.mult

    nc = bacc.Bacc("TRN2", target_bir_lowering=False, debug=False,
                   num_devices=N_CORES)

    xt_d = nc.dram_tensor("xt", [N_FEATS, SEQ], f16, kind="ExternalInput").ap()
    wqk_d = nc.dram_tensor("wqk", [N_FEATS, 256], f16, kind="ExternalInput").ap()
    wv_d = nc.dram_tensor("wv", [N_FEATS, 128], f16, kind="ExternalInput").ap()
    wo_d = nc.dram_tensor("wo", [128, N_FEATS], f16, kind="ExternalInput").ap()
    sel_d = nc.dram_tensor("sel", [2, 128], f32r, kind="ExternalInput").ap()
    id_d = nc.dram_tensor("ident", [128, 128], f16, kind="ExternalInput").ap()
    out_d = nc.dram_tensor("out", [N_FEATS, SEQ], f16, kind="ExternalOutput").ap()
    import os as _os
    _dbg = _os.environ.get("KDEBUG") == "1"
    if _dbg:
        dbg_vfull = nc.dram_tensor("dbg_vfull", [128, NJB, 2, 128], f16, kind="ExternalOutput").ap()
        dbg_q = nc.dram_tensor("dbg_q", [128, SEQ], f16, kind="ExternalOutput").ap()
        dbg_k = nc.dram_tensor("dbg_k", [128, SEQ], f16, kind="ExternalOutput").ap()
        dbg_dinv = nc.dram_tensor("dbg_dinv", [66, SEQ], f32, kind="ExternalOutput").ap()
        dbg_e = nc.dram_tensor("dbg_e", [128, W], f16, kind="ExternalOutput").ap()

    with tile.TileContext(nc) as tc:
        with tc.tile_pool(name="persist", bufs=1) as pp:
            wqk_sb = pp.tile([128, FCH, 256], f16)
            wv_sb = pp.tile([128, FCH, 128], f16)
            wo_sb = pp.tile([128, N_FEATS], f16)
            sel_sb = pp.tile([66, 128], f32r)
            id_sb = pp.tile([128, 128], f16)
            nc.gpsimd.dma_start(out=id_sb[:], in_=id_d[:])
            nc.gpsimd.dma_start(out=wqk_sb[:], in_=wqk_d.rearrange("(c p) m -> p c m", p=128))
            nc.gpsimd.dma_start(out=wv_sb[:], in_=wv_d.rearrange("(c p) m -> p c m", p=128))
            nc.gpsimd.dma_start(out=wo_sb[:], in_=wo_d[:])
            nc.gpsimd.dma_start(out=sel_sb[64:66, :], in_=sel_d[:])

            # warm the exp activation table while projections run
            scratch = pp.tile([1, 2], f32)
            nc.vector.memset(scratch[:], 0.0)
            nc.scalar.activation(out=scratch[:], in_=scratch[:], func=EXP)

            q_t = [pp.tile([128, 512], f16, name=f"q_t{i}") for i in range(NCH)]
            k_t = [pp.tile([128, 512], f16, name=f"k_t{i}") for i in range(NCH)]
            # rolled copies: A-head data duplicated into rows 64:128 (q only
            # for chunks 6,7 used by the A2 half; k for all chunks)
            q2_t = {i: pp.tile([128, 512], f16, name=f"q2_t{i}") for i in (6, 7)}
            k2_t = [pp.tile([128, 512], f16, name=f"k2_t{i}") for i in range(NCH)]
            vT_t = [pp.tile([128, 512], f16, name=f"vT_t{i}") for i in range(NCH)]
            # V in [j, d] layout: per key block, half A = [v|1] (65 used),
            # half B = [v|1|1] (66 used; double ones row -> den lands on PSUM
            # rows 64:66 so DVE ops keep a 32-aligned partition base)
            vfull = pp.tile([128, NJB, 2, 128], f16)  # halves padded to 256B for the x-bar M2S write path
            nc.gpsimd.memset(vfull[:, :, 0, 64:66], 1.0)
            nc.gpsimd.memset(vfull[:, :, 1, 64:66], 1.0)
            # softmax denominators, rows 64 (A) and 65 (B)
            den = pp.tile([66, SEQ], f32r)
            nc.gpsimd.memset(den[64:66, :].bitcast(f32), 1.0)

            # x^T streamed chunk-major so projections start after ~1/8 of x
            xt = pp.tile([128, FCH, SEQ], f16)
            xt_r = xt_d.rearrange("(c p) n -> p c n", p=128)
            for n in range(NCH):
                for k in range(FCH):
                    nc.gpsimd.dma_start(out=xt[:, k, n * 512:(n + 1) * 512],
                                        in_=xt_r[:, k, n * 512:(n + 1) * 512])

            ps_s_cm = tc.tile_pool(name="ps_s", space="PSUM", bufs=2)
            ps_s = ps_s_cm.__enter__()
            ps_pj_cm = tc.tile_pool(name="ps_pj", space="PSUM", bufs=2)
            ps_pj = ps_pj_cm.__enter__()
            pe_cm = tc.tile_pool(name="exps", bufs=10)
            pe = pe_cm.__enter__()
            ps_ov_cm = tc.tile_pool(name="ps_ov", bufs=1, space="PSUM")
            ps_ov = ps_ov_cm.__enter__()
            p3_cm = tc.tile_pool(name="ph3", bufs=1)
            p3 = p3_cm.__enter__()

            def proj_qk(nch):
                for which, dst in ((0, q_t[nch]), (1, k_t[nch])):
                    pj = ps_pj.tile([128, 512], f32, tag="pj", name=f"pj{'qk'[which]}{nch}")
                    for k in range(FCH):
                        nc.tensor.matmul(pj[:], wqk_sb[:, k, which * 128:(which + 1) * 128],
                                         xt[:, k, nch * 512:(nch + 1) * 512],
                                         start=(k == 0), stop=(k == FCH - 1))
                    nc.vector.tensor_copy(dst[:], pj[:])

            def proj_v(nch):
                pv = ps_pj.tile([128, 512], f32, tag="pj", name=f"pjv{nch}")
                for k in range(FCH):
                    nc.tensor.matmul(pv[:], wv_sb[:, k, :],
                                     xt[:, k, nch * 512:(nch + 1) * 512],
                                     start=(k == 0), stop=(k == FCH - 1))
                nc.vector.tensor_copy(vT_t[nch][:], pv[:])
                ptt = ps_pj.tile([128, 4, 128], f16, tag="pj", name=f"ptr{nch}")
                for qq in range(4):
                    jb = nch * 4 + qq
                    nc.tensor.transpose(ptt[:, qq, :],
                                        vT_t[nch][:, qq * 128:(qq + 1) * 128], id_sb[:])
                    nc.vector.tensor_copy(vfull[:, jb, 0, 0:64], ptt[:, qq, 0:64])
                    nc.vector.tensor_copy(vfull[:, jb, 1, 0:64], ptt[:, qq, 64:128])

            # chunks 0,1 up front; the rest interleave into pair 0's jb
            # loop (emission order sets scheduler priority - this lets the
            # exp stream start ~10us in instead of after all projections)
            for n in (0, 1):
                proj_qk(n)
                proj_v(n)

            def emit_proj(jb):
                n = jb // 4 + 2
                if jb % 4 == 0 and n < NCH:
                    proj_qk(n)
                    proj_v(n)
                if jb == NJB - 4:
                    # rolled copies for the A2 half (SWDGE; emitted late so
                    # they don't head-of-line-block the xt streaming DMAs)
                    for i in range(NCH):
                        nc.gpsimd.dma_start(out=k2_t[i][64:128, :], in_=k_t[i][0:64, :])
                    for i in (6, 7):
                        nc.gpsimd.dma_start(out=q2_t[i][64:128, :], in_=q_t[i][0:64, :])

            # pair = (A-chunk on PE rows 0:64, B/A2-chunk on rows 64:128),
            # each covering a 512-wide i-chunk; 512-wide accumulators keep
            # ov1+ov2 to 2 PSUM banks so projections get their own tag
            pairs = [(0, "B", 0), (1, "B", 1), (2, "B", 2), (3, "B", 3),
                     (4, "A2", 6), (5, "A2", 7)]

            for pi, (a_ch, kind, c_ch) in enumerate(pairs):
                ov1 = ps_ov.tile([65, 512], f32, tag="ov1", name=f"ov1_{pi}")
                m2 = 66 if kind == "B" else 65
                ov2 = ps_ov.tile([m2, 512], f32, tag="ov2", name=f"ov2_{pi}")
                q2 = q_t[c_ch] if kind == "B" else q2_t[c_ch]
                kt2 = k_t if kind == "B" else k2_t
                half2 = 1 if kind == "B" else 0
                for jb in range(NJB):
                    if pi == 0:
                        emit_proj(jb)
                    jc, jo = jb // 4, (jb % 4) * 128
                    sp = ps_s.tile([128, W], f32, tag="s", name=f"s{pi}_{jb}")
                    nc.tensor.matmul(sp[:, 0:512], k_t[jc][0:64, jo:jo + 128],
                                     q_t[a_ch][0:64, :], start=True, stop=True)
                    nc.tensor.matmul(sp[:, 512:1024], kt2[jc][64:128, jo:jo + 128],
                                     q2[64:128, :], start=True, stop=True)
                    e = pe.tile([128, W], f16, tag="e", name=f"e{pi}_{jb}")
                    nc.scalar.activation(out=e[:], in_=sp[:], func=EXP, scale=0.125)
                    if _dbg and pi == 0 and jb == 0:
                        nc.gpsimd.dma_start(out=dbg_e[:], in_=e[:])
                    nc.tensor.matmul(ov1[:], vfull[:, jb, 0, 0:65], e[:, 0:512],
                                     start=(jb == 0), stop=(jb == NJB - 1))
                    nc.tensor.matmul(ov2[:], vfull[:, jb, half2, 0:m2], e[:, 512:1024],
                                     start=(jb == 0), stop=(jb == NJB - 1))

                # ---- per-pair epilogue, overlapped with the next pair ----
                p1, p2 = a_ch * 512, c_ch * 512
                if kind == "B":
                    # B den into rows 64:66 first, then A den overwrites row 64
                    nc.vector.tensor_copy(den[64:66, p2:p2 + 512], ov2[64:66, :])
                else:
                    nc.vector.tensor_copy(den[64:65, p2:p2 + 512], ov2[64:65, :])
                nc.vector.tensor_copy(den[64:65, p1:p1 + 512], ov1[64:65, :])

                if kind == "B":
                    chunks = [(p1, ov1, ov2)]
                else:
                    chunks = [(p1, ov1, None), (p2, ov2, None)]
                nms = []
                for cb, (c0, ovA, ovB) in enumerate(chunks):
                    bc = ps_s.tile([128, 512], f32, tag="s", name=f"bc{pi}_{cb}")
                    nc.tensor.matmul(bc[:], sel_sb[64:66, :], den[64:66, c0:c0 + 512],
                                     start=True, stop=True)
                    rows = 128 if ovB is not None else 64
                    rc = p3.tile([rows, 512], f32, tag="rc", name=f"rc{pi}_{cb}", bufs=3)
                    nc.vector.reciprocal_approx_fast(out=rc[:], in_=bc[0:rows, :])
                    nm = p3.tile([rows, 512], f16, tag="nm", name=f"nm{pi}_{cb}", bufs=5)
                    nc.vector.scalar_tensor_tensor(
                        out=nm[0:64, :], in0=ovA[0:64, :], scalar=0.0,
                        in1=rc[0:64, :], op0=BYP, op1=MUL)
                    if ovB is not None:
                        nc.vector.scalar_tensor_tensor(
                            out=nm[64:128, :], in0=ovB[0:64, :], scalar=0.0,
                            in1=rc[64:128, :], op0=BYP, op1=MUL)
                    nms.append((c0, rows, nm))
                pon = 0
                for cb, (c0, rows, nm) in enumerate(nms):
                    for fb in range(FCH):
                        po = ps_ov.tile([128, 512], f32, tag=("ov1", "ov2")[pon % 2],
                                        name=f"po{pi}_{cb}_{fb}")
                        pon += 1
                        nc.tensor.matmul(po[:], wo_sb[0:rows, fb * 128:(fb + 1) * 128],
                                         nm[:], start=True, stop=True)
                        ob = p3.tile([128, 512], f16, tag="ob", name=f"ob{pi}_{cb}_{fb}",
                                     bufs=4)
                        if pi < 4:
                            nc.vector.tensor_copy(ob[:], po[:])
                        else:
                            nc.scalar.copy(ob[:], po[:])
                        nc.sync.dma_start(out=out_d[fb * 128:(fb + 1) * 128, c0:c0 + 512],
                                          in_=ob[:])

            if _dbg:
                nc.gpsimd.dma_start(out=dbg_dinv[:], in_=den[:].bitcast(f32))
            p3_cm.__exit__(None, None, None)
            ps_ov_cm.__exit__(None, None, None)
            pe_cm.__exit__(None, None, None)
            ps_pj_cm.__exit__(None, None, None)
            ps_s_cm.__exit__(None, None, None)

    nc.compile()
    return nc


def _get_program():
    global _PROGRAM
    if _PROGRAM is None:
        _PROGRAM = _build_program()
    return _PROGRAM


def kernel(x: np.ndarray, w_qkv: np.ndarray, w_out: np.ndarray) -> np.ndarray:
    global LAST_RESULT
    import os
    try:
        import antenv.axon_hooks  # noqa: F401
    except ImportError:
        # without the NTFF hook, a leaked BASS_TRACE=1 would crash the
        # axon trace path inside run_bass_kernel_spmd
        os.environ["BASS_NEVER_TRACE"] = "1"
    from concourse.bass_utils import run_bass_kernel_spmd

    nc = _get_program()
    x2 = np.ascontiguousarray(x[0], dtype=np.float32)          # [SEQ, F]
    w_qkv = np.asarray(w_qkv, dtype=np.float32)                # [2304, F]
    w_out = np.asarray(w_out, dtype=np.float32)                # [F, 768]

    # per-head slices of w_qkv rows: o = h*192 + d*3 + {0:q, 1:k, 2:v}
    def wslice(h, which):
        return w_qkv[h * 192 + which:(h + 1) * 192:3, :]       # [64, F]

    sel = np.zeros((2, 128), dtype=np.float32)
    sel[0, 0:64] = 1.0
    sel[1, 64:128] = 1.0
    ident = np.eye(128, dtype=np.float16)

    xt_plain = np.ascontiguousarray(x2.T.astype(np.float16))   # [F, SEQ]
    xt_rolled = np.ascontiguousarray(np.roll(x2, -SEQ // 2, axis=0).T.astype(np.float16))

    in_maps = []
    rolls = []
    for c in range(N_CORES):
        hA = c
        hB = 8 + c // 2
        roll = (SEQ // 2) if (c % 2) else 0
        rolls.append(roll)
        wqk = np.ascontiguousarray(np.concatenate(
            [wslice(hA, 0), wslice(hB, 0), wslice(hA, 1), wslice(hB, 1)],
            axis=0).T.astype(np.float16))
        wv = np.ascontiguousarray(np.concatenate(
            [wslice(hA, 2), wslice(hB, 2)], axis=0).T.astype(np.float16))
        cols = list(range(hA * 64, hA * 64 + 64)) + list(range(hB * 64, hB * 64 + 64))
        wo = np.ascontiguousarray(w_out[:, cols].T.astype(np.float16))  # [128, F]
        in_maps.append({
            "xt": xt_rolled if roll else xt_plain,
            "wqk": wqk, "wv": wv, "wo": wo, "sel": sel, "ident": ident,
        })

    res = run_bass_kernel_spmd(nc, in_maps, list(range(N_CORES)))
    LAST_RESULT = res

    acc = np.zeros((SEQ, N_FEATS), dtype=np.float64)
    for c in range(N_CORES):
        part = res.results[c]["out"].astype(np.float32)        # [F, SEQ]
        if rolls[c]:
            part = np.roll(part, rolls[c], axis=1)
        acc += part.T.astype(np.float64)
    return acc.astype(np.float32)[None]


# revision 23
# speedup vs baseline: 1.3287x; 1.2545x over previous
"""Trainium2 Bass kernel for 12-head attention (SEQ=4096, D=768), 8-core SPMD.

Sharding: head-parallel with a sequence-split remainder. Core c owns full head
A_c = c and the half of head B_c = 8 + c//2 selected by (c % 2). Upper-half
cores receive a sequence-rolled copy of x so every core's program computes
local queries [0, 2048) for its B head (pure SPMD, no divergent control flow).
Each core returns a partial output projection [768, 4096] (fp16); the host
un-rolls, transposes and sums the 8 partials.

Per-core dataflow (matmuls in fp16, softmax internals in fp32):
  x^T streamed chunk-major -> QKV^T projections (start attention early)
  -> scores S^T[j,i] = K^T(lhsT) x Q^T(rhs) with the pair's two heads on
  disjoint PE row groups -> exp on ScalarE (scale=1/8 folded; scores within
  [-10, 10]) -> attn@V with [V | ones] as stationary operand (denominators
  fall out as PSUM row 64); V reaches [j, d] layout via DMA x-bar transpose
  -> per-pair epilogue overlapped with the next pair's attention: reciprocal
  of the denominator rows straight from PSUM, K=2 broadcast matmul, fused
  drain-normalize (scalar_tensor_tensor), output projection into the freed
  accumulator banks, fp16 DMA out.
"""

import numpy as np

N_CORES = 8
N_HEADS = 12
HEAD_DIM = 64
N_FEATS = 768
SEQ = 4096
FCH = N_FEATS // 128  # contraction chunks of the feature dim
W = 1024              # i-chunk width (exp granularity)
NJB = SEQ // 128      # key blocks
NH = W // 512         # 512-wide matmul sub-chunks per i-chunk
NCH = SEQ // 512      # 512-wide seq chunks

_PROGRAM = None
LAST_RESULT = None


def _build_program():
    import concourse.tile as tile
    from concourse import bacc, mybir

    f32 = mybir.dt.float32
    f32r = mybir.dt.float32r
    f16 = mybir.dt.float16
    EXP = mybir.ActivationFunctionType.Exp
    BYP = mybir.AluOpType.bypass
    MUL = mybir.AluOpType.mult

    nc = bacc.Bacc("TRN2", target_bir_lowering=False, debug=False,
                   num_devices=N_CORES)

    xt_d = nc.dram_tensor("xt", [N_FEATS, SEQ], f16, kind="ExternalInput").ap()
    wqk_d = nc.dram_tensor("wqk", [N_FEATS, 256], f16, kind="ExternalInput").ap()
    wv_d = nc.dram_tensor("wv", [N_FEATS, 128], f16, kind="ExternalInput").ap()
    wo_d = nc.dram_tensor("wo", [128, N_FEATS], f16, kind="ExternalInput").ap()
    sel_d = nc.dram_tensor("sel", [2, 128], f32r, kind="ExternalInput").ap()
    id_d = nc.dram_tensor("ident", [128, 128], f16, kind="ExternalInput").ap()
    out_d = nc.dram_tensor("out", [N_FEATS, SEQ], f16, kind="ExternalOutput").ap()
    import os as _os
    _dbg = _os.environ.get("KDEBUG") == "1"
    if _dbg:
        dbg_vfull = nc.dram_tensor("dbg_vfull", [128, NJB, 2, 128], f16, kind="ExternalOutput").ap()
        dbg_q = nc.dram_tensor("dbg_q", [128, SEQ], f16, kind="ExternalOutput").ap()
        dbg_k = nc.dram_tensor("dbg_k", [128, SEQ], f16, kind="ExternalOutput").ap()
        dbg_dinv = nc.dram_tensor("dbg_dinv", [66, SEQ], f32, kind="ExternalOutput").ap()
        dbg_e = nc.dram_tensor("dbg_e", [128, W], f16, kind="ExternalOutput").ap()

    with tile.TileContext(nc) as tc:
        with tc.tile_pool(name="persist", bufs=1) as pp:
            wqk_sb = pp.tile([128, FCH, 256], f16)
            wv_sb = pp.tile([128, FCH, 128], f16)
            wo_sb = pp.tile([128, N_FEATS], f16)
            sel_sb = pp.tile([66, 128], f32r)
            id_sb = pp.tile([128, 128], f16)
            nc.gpsimd.dma_start(out=wqk_sb[:], in_=wqk_d.rearrange("(c p) m -> p c m", p=128))

            # warm the exp activation table while projections run
            scratch = pp.tile([1, 2], f32)
            nc.vector.memset(scratch[:], 0.0)
            nc.scalar.activation(out=scratch[:], in_=scratch[:], func=EXP)

            q_t = [pp.tile([128, 512], f16, name=f"q_t{i}") for i in range(NCH)]
            k_t = [pp.tile([128, 512], f16, name=f"k_t{i}") for i in range(NCH)]
            # rolled copies: A-head data duplicated into rows 64:128 (q only
            # for chunks 6,7 used by the A2 half; k for all chunks)
            q2_t = {i: pp.tile([128, 512], f16, name=f"q2_t{i}") for i in (6, 7)}
            k2_t = [pp.tile([128, 512], f16, name=f"k2_t{i}") for i in range(NCH)]
            vT_t = [pp.tile([128, 512], f16, name=f"vT_t{i}") for i in range(NCH)]
            # V in [j, d] layout: per key block, half A = [v|1] (65 used),
            # half B = [v|1|1] (66 used; double ones row -> den lands on PSUM
            # rows 64:66 so DVE ops keep a 32-aligned partition base)
            vfull = pp.tile([128, NJB, 2, 128], f16)  # halves padded to 256B for the x-bar M2S write path
            # softmax denominators, rows 64 (A) and 65 (B)
            den = pp.tile([66, SEQ], f32r)

            # x^T streamed chunk-major so projections start after ~1/8 of x
            xt = pp.tile([128, FCH, SEQ], f16)
            xt_r = xt_d.rearrange("(c p) n -> p c n", p=128)
            for n in range(NCH):
                for k in range(FCH):
                    nc.gpsimd.dma_start(out=xt[:, k, n * 512:(n + 1) * 512],
                                        in_=xt_r[:, k, n * 512:(n + 1) * 512])
                if n == 1:
                    # deferred loads/inits: not needed for the first scores,
                    # so they stay off the critical path
                    nc.gpsimd.dma_start(out=wv_sb[:], in_=wv_d.rearrange("(c p) m -> p c m", p=128))
                    nc.gpsimd.dma_start(out=id_sb[:], in_=id_d[:])
                    nc.gpsimd.dma_start(out=wo_sb[:], in_=wo_d[:])
                    nc.gpsimd.dma_start(out=sel_sb[64:66, :], in_=sel_d[:])
                    nc.gpsimd.memset(vfull[:, :, 0, 64:66], 1.0)
                    nc.gpsimd.memset(vfull[:, :, 1, 64:66], 1.0)
                    nc.gpsimd.memset(den[64:66, :].bitcast(f32), 1.0)

            ps_s_cm = tc.tile_pool(name="ps_s", space="PSUM", bufs=2)
            ps_s = ps_s_cm.__enter__()
            ps_pj_cm = tc.tile_pool(name="ps_pj", space="PSUM", bufs=2)
            ps_pj = ps_pj_cm.__enter__()
            pe_cm = tc.tile_pool(name="exps", bufs=10)
            pe = pe_cm.__enter__()
            ps_ov_cm = tc.tile_pool(name="ps_ov", bufs=1, space="PSUM")
            ps_ov = ps_ov_cm.__enter__()
            p3_cm = tc.tile_pool(name="ph3", bufs=1)
            p3 = p3_cm.__enter__()

            def proj_qk(nch):
                for which, dst in ((0, q_t[nch]), (1, k_t[nch])):
                    pj = ps_pj.tile([128, 512], f32, tag="pj", name=f"pj{'qk'[which]}{nch}")
                    for k in range(FCH):
                        nc.tensor.matmul(pj[:], wqk_sb[:, k, which * 128:(which + 1) * 128],
                                         xt[:, k, nch * 512:(nch + 1) * 512],
                                         start=(k == 0), stop=(k == FCH - 1))
                    nc.vector.tensor_copy(dst[:], pj[:])

            def proj_v(nch):
                pv = ps_pj.tile([128, 512], f32, tag="pj", name=f"pjv{nch}")
                for k in range(FCH):
                    nc.tensor.matmul(pv[:], wv_sb[:, k, :],
                                     xt[:, k, nch * 512:(nch + 1) * 512],
                                     start=(k == 0), stop=(k == FCH - 1))
                nc.vector.tensor_copy(vT_t[nch][:], pv[:])
                ptt = ps_pj.tile([128, 4, 128], f16, tag="pj", name=f"ptr{nch}")
                for qq in range(4):
                    jb = nch * 4 + qq
                    nc.tensor.transpose(ptt[:, qq, :],
                                        vT_t[nch][:, qq * 128:(qq + 1) * 128], id_sb[:])
                    nc.vector.tensor_copy(vfull[:, jb, 0, 0:64], ptt[:, qq, 0:64])
                    nc.vector.tensor_copy(vfull[:, jb, 1, 0:64], ptt[:, qq, 64:128])

            # chunks 0,1 up front; the rest interleave into pair 0's jb
            # loop (emission order sets scheduler priority - this lets the
            # exp stream start ~10us in instead of after all projections)
            for n in (0, 1):
                proj_qk(n)
                proj_v(n)

            def emit_proj(jb):
                n = jb // 4 + 2
                if jb % 4 == 0 and n < NCH:
                    proj_qk(n)
                    proj_v(n)
                if jb == NJB - 4:
                    # rolled copies for the A2 half (SWDGE; emitted late so
                    # they don't head-of-line-block the xt streaming DMAs)
                    for i in range(NCH):
                        nc.gpsimd.dma_start(out=k2_t[i][64:128, :], in_=k_t[i][0:64, :])
                    for i in (6, 7):
                        nc.gpsimd.dma_start(out=q2_t[i][64:128, :], in_=q_t[i][0:64, :])

            # pair = (A-chunk on PE rows 0:64, B/A2-chunk on rows 64:128),
            # each covering a 512-wide i-chunk; 512-wide accumulators keep
            # ov1+ov2 to 2 PSUM banks so projections get their own tag
            pairs = [(0, "B", 0), (1, "B", 1), (2, "B", 2), (3, "B", 3),
                     (4, "A2", 6), (5, "A2", 7)]

            for pi, (a_ch, kind, c_ch) in enumerate(pairs):
                ov1 = ps_ov.tile([65, 512], f32, tag="ov1", name=f"ov1_{pi}")
                m2 = 66 if kind == "B" else 65
                ov2 = ps_ov.tile([m2, 512], f32, tag="ov2", name=f"ov2_{pi}")
                q2 = q_t[c_ch] if kind == "B" else q2_t[c_ch]
                kt2 = k_t if kind == "B" else k2_t
                half2 = 1 if kind == "B" else 0
                for jb in range(NJB):
                    if pi == 0:
                        emit_proj(jb)
                    jc, jo = jb // 4, (jb % 4) * 128
                    sp = ps_s.tile([128, W], f32, tag="s", name=f"s{pi}_{jb}")
                    nc.tensor.matmul(sp[:, 0:512], k_t[jc][0:64, jo:jo + 128],
                                     q_t[a_ch][0:64, :], start=True, stop=True)
                    nc.tensor.matmul(sp[:, 512:1024], kt2[jc][64:128, jo:jo + 128],
                                     q2[64:128, :], start=True, stop=True)
                    e = pe.tile([128, W], f16, tag="e", name=f"e{pi}_{jb}")
                    nc.scalar.activation(out=e[:], in_=sp[:], func=EXP, scale=0.125)
                    if _dbg and pi == 0 and jb == 0:
                        nc.gpsimd.dma_start(out=dbg_e[:], in_=e[:])
                    nc.tensor.matmul(ov1[:], vfull[:, jb, 0, 0:65], e[:, 0:512],
                                     start=(jb == 0), stop=(jb == NJB - 1))
                    nc.tensor.matmul(ov2[:], vfull[:, jb, half2, 0:m2], e[:, 512:1024],
                                     start=(jb == 0), stop=(jb == NJB - 1))

                # ---- per-pair epilogue, overlapped with the next pair ----
                p1, p2 = a_ch * 512, c_ch * 512
                if kind == "B":
                    # B den into rows 64:66 first, then A den overwrites row 64
                    nc.vector.tensor_copy(den[64:66, p2:p2 + 512], ov2[64:66, :])
                else:
                    nc.vector.tensor_copy(den[64:65, p2:p2 + 512], ov2[64:65, :])
                nc.vector.tensor_copy(den[64:65, p1:p1 + 512], ov1[64:65, :])

                if kind == "B":
                    chunks = [(p1, ov1, ov2)]
                else:
                    chunks = [(p1, ov1, None), (p2, ov2, None)]
                nms = []
                for cb, (c0, ovA, ovB) in enumerate(chunks):
                    bc = ps_pj.tile([128, 512], f32, tag="pj", name=f"bc{pi}_{cb}")
                    nc.tensor.matmul(bc[:], sel_sb[64:66, :], den[64:66, c0:c0 + 512],
                                     start=True, stop=True)
                    rows = 128 if ovB is not None else 64
                    rc = p3.tile([rows, 512], f32, tag="rc", name=f"rc{pi}_{cb}", bufs=3)
                    nc.vector.reciprocal_approx_fast(out=rc[:], in_=bc[0:rows, :])
                    nm = p3.tile([rows, 512], f16, tag="nm", name=f"nm{pi}_{cb}", bufs=5)
                    nc.vector.scalar_tensor_tensor(
                        out=nm[0:64, :], in0=ovA[0:64, :], scalar=0.0,
                        in1=rc[0:64, :], op0=BYP, op1=MUL)
                    if ovB is not None:
                        nc.vector.scalar_tensor_tensor(
                            out=nm[64:128, :], in0=ovB[0:64, :], scalar=0.0,
                            in1=rc[64:128, :], op0=BYP, op1=MUL)
                    nms.append((c0, rows, nm))
                pon = 0
                for cb, (c0, rows, nm) in enumerate(nms):
                    for fb in range(FCH):
                        po = ps_ov.tile([128, 512], f32, tag=("ov1", "ov2")[pon % 2],
                                        name=f"po{pi}_{cb}_{fb}")
                        pon += 1
                        nc.tensor.matmul(po[:], wo_sb[0:rows, fb * 128:(fb + 1) * 128],
                                         nm[:], start=True, stop=True)
                        ob = p3.tile([128, 512], f16, tag="ob", name=f"ob{pi}_{cb}_{fb}",
                                     bufs=4)
                        if pi < 4 or pon % 2 == 0:
                            nc.vector.tensor_copy(ob[:], po[:])
                        else:
                            nc.scalar.copy(ob[:], po[:])
                        nc.sync.dma_start(out=out_d[fb * 128:(fb + 1) * 128, c0:c0 + 512],
                                          in_=ob[:])

            if _dbg:
                nc.gpsimd.dma_start(out=dbg_dinv[:], in_=den[:].bitcast(f32))
            p3_cm.__exit__(None, None, None)
            ps_ov_cm.__exit__(None, None, None)
            pe_cm.__exit__(None, None, None)
            ps_pj_cm.__exit__(None, None, None)
            ps_s_cm.__exit__(None, None, None)

    nc.compile()
    return nc


def _get_program():
    global _PROGRAM
    if _PROGRAM is None:
        _PROGRAM = _build_program()
    return _PROGRAM


def kernel(x: np.ndarray, w_qkv: np.ndarray, w_out: np.ndarray) -> np.ndarray:
    global LAST_RESULT
    import os
    try:
        import antenv.axon_hooks  # noqa: F401
    except ImportError:
        # without the NTFF hook, a leaked BASS_TRACE=1 would crash the
        # axon trace path inside run_bass_kernel_spmd
        os.environ["BASS_NEVER_TRACE"] = "1"
    from concourse.bass_utils import run_bass_kernel_spmd

    nc = _get_program()
    x2 = np.ascontiguousarray(x[0], dtype=np.float32)          # [SEQ, F]
    w_qkv = np.asarray(w_qkv, dtype=np.float32)                # [2304, F]
    w_out = np.asarray(w_out, dtype=np.float32)                # [F, 768]

    # per-head slices of w_qkv rows: o = h*192 + d*3 + {0:q, 1:k, 2:v}
    def wslice(h, which):
        return w_qkv[h * 192 + which:(h + 1) * 192:3, :]       # [64, F]

    sel = np.zeros((2, 128), dtype=np.float32)
    sel[0, 0:64] = 1.0
    sel[1, 64:128] = 1.0
    ident = np.eye(128, dtype=np.float16)

    xt_plain = np.ascontiguousarray(x2.T.astype(np.float16))   # [F, SEQ]
    xt_rolled = np.ascontiguousarray(np.roll(x2, -SEQ // 2, axis=0).T.astype(np.float16))

    in_maps = []
    rolls = []
    for c in range(N_CORES):
        hA = c
        hB = 8 + c // 2
        roll = (SEQ // 2) if (c % 2) else 0
        rolls.append(roll)
        wqk = np.ascontiguousarray(np.concatenate(
            [wslice(hA, 0), wslice(hB, 0), wslice(hA, 1), wslice(hB, 1)],
            axis=0).T.astype(np.float16))
        wv = np.ascontiguousarray(np.concatenate(
            [wslice(hA, 2), wslice(hB, 2)], axis=0).T.astype(np.float16))
        cols = list(range(hA * 64, hA * 64 + 64)) + list(range(hB * 64, hB * 64 + 64))
        wo = np.ascontiguousarray(w_out[:, cols].T.astype(np.float16))  # [128, F]
        in_maps.append({
            "xt": xt_rolled if roll else xt_plain,
            "wqk": wqk, "wv": wv, "wo": wo, "sel": sel, "ident": ident,
        })

    res = run_bass_kernel_spmd(nc, in_maps, list(range(N_CORES)))
    LAST_RESULT = res

    acc = np.zeros((SEQ, N_FEATS), dtype=np.float64)
    for c in range(N_CORES):
        part = res.results[c]["out"].astype(np.float32)        # [F, SEQ]
        if rolls[c]:
            part = np.roll(part, rolls[c], axis=1)
        acc += part.T.astype(np.float64)
    return acc.astype(np.float32)[None]
